# revision 18
# baseline (speedup 1.0000x reference)
"""CRF loss on 8 Trainium2 cores — sequence-sharded relay scan.

The partition function is log(1^T M x0) with M = A_1023 ... A_1,
A_s = diag(w_s) E^T (probability space, E = exp(trans) * 2^-9 prescaled,
w_s = exp(emissions_s)). Products of positive matrices contract
projectively (Birkhoff), so each core owns a 128-step sequence block,
split into 4 chains that each START K=8 steps EARLY from a uniform
anchor: after the warmup the state direction matches the true incoming
state to ~1e-7, and the per-chain log-gains ln(1^T end) - ln(1^T after
warmup) telescope exactly to the full partition (anchor scale cancels).

Per core: 4 chains x 40 supersteps over state [T=128, B=256], one PE
matmul + one evacuation multiply (DVE or GPSIMD, interleaved) per step
in bf16 with fp32 PSUM. Chain 0 of core 0 warms up with an identity
transition matrix (input transW) + zero emissions so it starts exactly
at onehot(START). Core 7's chain 3 has a phantom last step (s=1024,
zero emissions); the host uses its step-38 snapshot instead.

Gold-path score: per-batch sums of em[b,s,tag] and trans[pairs] are
reduced on-device from host-relaid index-gathered value tensors (pure
take_along_axis moves; all arithmetic on device). Host only adds the
per-core scalar outputs.
"""

import math
import sys

import numpy as np

sys.path.insert(0, "/opt/trn_rl_repo")

import concourse.bacc as bacc_mod
import concourse.bass as bass
import concourse.mybir as mybir
import concourse.tile as tile
from concourse.bass_utils import run_bass_kernel_spmd

import ml_dtypes

B, S, T = 256, 1024, 128
NCORES = 8
START, END = T - 2, T - 1          # 126, 127
K = 8                              # warmup steps per chain
NSUP = 40                          # supersteps per chain (K + 32)
NCH = 4                            # chains per core
ROWS = 136                         # emT rows per core
PRE = 9.0                          # 2^-9 prescale on E
BIAS0 = float(-PRE * math.log(2.0))
SC = float(2.0**40)                # anchor scale
ESLOT = 129                        # gold slots per batch element
F32 = mybir.dt.float32
BF16 = mybir.dt.bfloat16
U16 = mybir.dt.uint16
NSL = 9                            # emT stream slices (8x16 rows + 1x8)
SLICE_ROWS = [16] * 8 + [8]
SLICE_ORDER = [0, 2, 4, 6, 1, 3, 5, 7, 8]
POOL_STEPS = (1, 3, 5)             # j%8 in this set -> evac on GPSIMD
NSNAP = 9                          # 4 m's, 3 e's, eD38, eD39


def _build_kernel() -> bass.Bass:
    nc = bacc_mod.Bacc()
    emT_d = nc.dram_tensor("emT", [T, ROWS, B], BF16, kind="ExternalInput")
    trans_d = nc.dram_tensor("trans", [T, T], F32, kind="ExternalInput")
    transW_d = nc.dram_tensor("transW", [T, T], F32, kind="ExternalInput")
    x0_d = nc.dram_tensor("x0", [T, 2 * B], BF16, kind="ExternalInput")
    emG_d = nc.dram_tensor("emG", [T, 2 * ESLOT], BF16, kind="ExternalInput")
    trG_d = nc.dram_tensor("trG", [T, 2 * ESLOT], BF16, kind="ExternalInput")
    out_lnz = nc.dram_tensor("lnz", [1, NSNAP * B], F32, kind="ExternalOutput")
    out_gE = nc.dram_tensor("goldE", [T, 2], F32, kind="ExternalOutput")
    out_gT = nc.dram_tensor("goldT", [T, 2], F32, kind="ExternalOutput")

    Exp = mybir.ActivationFunctionType.Exp
    Ln = mybir.ActivationFunctionType.Ln
    AX = mybir.AxisListType.X

    with tile.TileContext(nc) as tc:
        with (
            tc.tile_pool(name="constp", bufs=1) as constp,
            tc.tile_pool(name="wpool", bufs=1) as wpool,
            tc.tile_pool(name="rawp", bufs=3) as rawp,
            tc.tile_pool(name="statep", bufs=2) as statep,
            tc.tile_pool(name="goldp", bufs=1) as goldp,
            tc.tile_pool(name="miscp", bufs=1) as miscp,
            tc.tile_pool(name="psump", bufs=1, space="PSUM") as psump,
            tc.tile_pool(name="psums", bufs=2, space="PSUM") as psums,
        ):
            # ---- constants ----
            trans_t = constp.tile([T, T], F32)
            nc.sync.dma_start(out=trans_t[:], in_=trans_d[:, :])
            transW_t = constp.tile([T, T], F32)
            nc.sync.dma_start(out=transW_t[:], in_=transW_d[:, :])
            bias0_t = constp.tile([T, 1], F32)
            nc.vector.memset(bias0_t[:], BIAS0)
            zero_t = constp.tile([T, 1], F32)
            nc.vector.memset(zero_t[:], 0.0)
            Ep = constp.tile([T, T], BF16)
            nc.scalar.activation(Ep[:], trans_t[:], Exp, bias=bias0_t[:])
            EpW = constp.tile([T, T], BF16)
            nc.scalar.activation(EpW[:], transW_t[:], Exp, bias=bias0_t[:])
            ones1 = constp.tile([T, 1], BF16)
            nc.vector.memset(ones1[:], 1.0)
            # host-gathered gold values (emissions + transitions)
            emG_t = constp.tile([T, 2 * ESLOT], BF16)
            nc.gpsimd.dma_start(out=emG_t[:], in_=emG_d[:, :])
            trG_t = constp.tile([T, 2 * ESLOT], BF16)
            nc.gpsimd.dma_start(out=trG_t[:], in_=trG_d[:, :])
            # initial states: pair tiles [chainA | chainB], [chainC | chainD]
            Xab = constp.tile([T, 2 * B], BF16)
            nc.sync.dma_start(out=Xab[:], in_=x0_d[:, :])
            Xcd = constp.tile([T, 2 * B], BF16)
            nc.vector.memset(Xcd[:], SC)

            # ---- emissions streaming + exp into PAIRED w layout ----
            # wAB/wCD[:, j, h, :] = w for pair-half h's superstep j
            wAB = wpool.tile([T, NSUP * 2 * B], BF16)
            wCD = wpool.tile([T, NSUP * 2 * B], BF16)
            wABv = wAB[:].rearrange("p (j h b) -> p j h b", j=NSUP, h=2)
            wCDv = wCD[:].rearrange("p (j h b) -> p j h b", j=NSUP, h=2)
            # per slice k: (dsts) each = (view, j0, j1, half, row_off_in_slice)
            SLICE_DSTS = {
                0: [(wABv, 0, 16, 0, 0)],
                1: [(wABv, 16, 32, 0, 0)],
                2: [(wABv, 0, 16, 1, 0), (wABv, 32, 40, 0, 0)],
                3: [(wABv, 16, 32, 1, 0)],
                4: [(wCDv, 0, 16, 0, 0), (wABv, 32, 40, 1, 0)],
                5: [(wCDv, 16, 32, 0, 0)],
                6: [(wCDv, 0, 16, 1, 0), (wCDv, 32, 40, 0, 0)],
                7: [(wCDv, 16, 32, 1, 0)],
                8: [(wCDv, 32, 40, 1, 0)],
            }
            row0 = [0, 16, 32, 48, 64, 80, 96, 112, 128]
            for n, k in enumerate(SLICE_ORDER):
                nr = SLICE_ROWS[k]
                raw = rawp.tile([T, 16 * B], BF16, tag="raw")
                src = emT_d[:, row0[k] : row0[k] + nr, :].rearrange("t s b -> t (s b)")
                qeng = nc.sync if n % 2 == 0 else nc.scalar
                qeng.dma_start(out=raw[:, 0 : nr * B], in_=src)
                for view, j0, j1, h, roff in SLICE_DSTS[k]:
                    nrows = j1 - j0
                    nc.scalar.activation(
                        view[:, j0:j1, h, :],
                        raw[:, roff * B : (roff + nrows) * B],
                        Exp,
                        bias=zero_t[:],
                    )

            # ---- gold: device reduces of host-gathered values ----
            gered = goldp.tile([T, 2], F32)
            nc.vector.reduce_sum(
                out=gered[:], in_=emG_t[:].rearrange("p (h i) -> p h i", h=2), axis=AX
            )
            nc.sync.dma_start(out=out_gE[:, :], in_=gered[:])
            tgred = goldp.tile([T, 2], F32)
            nc.vector.reduce_sum(
                out=tgred[:], in_=trG_t[:].rearrange("p (h i) -> p h i", h=2), axis=AX
            )
            nc.sync.dma_start(out=out_gT[:, :], in_=tgred[:])

            # ---- the four relay chains ----
            snaps = miscp.tile([1, NSNAP * B], F32)

            def snapshot(Xsl, slot):
                ps = psums.tile([1, B], F32, tag="snap")
                nc.tensor.matmul(out=ps[:], lhsT=ones1[:], rhs=Xsl, start=True, stop=True)
                nc.vector.tensor_copy(out=snaps[:, slot * B : (slot + 1) * B], in_=ps[:])

            for j in range(NSUP):
                lhsA = EpW if j < K else Ep
                pab = psump.tile([T, 2 * B], F32, tag="pab")
                nc.tensor.matmul(out=pab[:, 0:B], lhsT=lhsA[:], rhs=Xab[:, 0:B],
                                 start=True, stop=True, skip_group_check=True)
                nc.tensor.matmul(out=pab[:, B : 2 * B], lhsT=Ep[:], rhs=Xab[:, B : 2 * B],
                                 start=True, stop=True, skip_group_check=True)
                Xab_n = statep.tile([T, 2 * B], BF16, tag="Xab")
                nc.vector.tensor_mul(
                    out=Xab_n[:], in0=wAB[:, j * 2 * B : (j + 1) * 2 * B], in1=pab[:]
                )
                Xab = Xab_n
                pcd = psump.tile([T, 2 * B], F32, tag="pcd")
                nc.tensor.matmul(out=pcd[:, 0:B], lhsT=Ep[:], rhs=Xcd[:, 0:B],
                                 start=True, stop=True, skip_group_check=True)
                nc.tensor.matmul(out=pcd[:, B : 2 * B], lhsT=Ep[:], rhs=Xcd[:, B : 2 * B],
                                 start=True, stop=True, skip_group_check=True)
                Xcd_n = statep.tile([T, 2 * B], BF16, tag="Xcd")
                nc.vector.tensor_mul(
                    out=Xcd_n[:], in0=wCD[:, j * 2 * B : (j + 1) * 2 * B], in1=pcd[:]
                )
                Xcd = Xcd_n
                if j == K - 1:
                    snapshot(Xab[:, 0:B], 0)
                    snapshot(Xab[:, B : 2 * B], 1)
                    snapshot(Xcd[:, 0:B], 2)
                    snapshot(Xcd[:, B : 2 * B], 3)
                elif j == NSUP - 2:
                    snapshot(Xcd[:, B : 2 * B], 7)             # eD38
                elif j == NSUP - 1:
                    snapshot(Xab[:, 0:B], 4)
                    snapshot(Xab[:, B : 2 * B], 5)
                    snapshot(Xcd[:, 0:B], 6)
                    snapshot(Xcd[:, B : 2 * B], 8)             # eD39

            lnz = miscp.tile([1, NSNAP * B], F32)
            nc.scalar.activation(lnz[:], snaps[:], Ln, bias=zero_t[0:1, :])
            nc.sync.dma_start(out=out_lnz[:, :], in_=lnz[:])

    nc.compile()
    return nc


def _bf16(x: np.ndarray) -> np.ndarray:
    return x.astype(ml_dtypes.bfloat16)


def _host_prep(emissions, tags, transitions):
    """Per-core input maps. Host does index/layout relaying only."""
    em32 = np.asarray(emissions, dtype=np.float32)
    emB = _bf16(em32)                                             # [B, S, T]
    tags = np.asarray(tags).astype(np.int64)
    trans = np.ascontiguousarray(np.asarray(transitions, dtype=np.float32))

    padded = np.concatenate(
        [np.full((B, 1), START, np.int64), tags, np.full((B, 1), END, np.int64)],
        axis=1,
    )
    prev_all, cur_all = padded[:, :-1], padded[:, 1:]             # pairs i=0..S

    transW0 = np.full((T, T), -1e4, np.float32)
    np.fill_diagonal(transW0, PRE * math.log(2.0))

    x0_0 = np.full((T, 2 * B), SC, np.float32)
    x0_0[:, 0:B] = 0.0
    x0_0[START, 0:B] = SC
    x0_u = np.full((T, 2 * B), SC, np.float32)

    bidx = np.arange(B)
    in_maps = []
    for c in range(NCORES):
        emT = np.zeros((T, ROWS, B), ml_dtypes.bfloat16)
        lo = max(1, 128 * c - 7)
        hi = min(S - 1, 128 * c + 128)
        rr = np.arange(lo, hi + 1) - (128 * c - 7)                # target rows
        emT[:, rr, :] = emB[:, lo : hi + 1, :].transpose(2, 1, 0)

        if c == 0:
            s_list = [0] + list(range(1, 129))
            i_list = list(range(0, 129))
        elif c == 7:
            s_list = list(range(897, 1024))
            i_list = list(range(897, 1024)) + [1024]
        else:
            s_list = list(range(128 * c + 1, 128 * c + 129))
            i_list = list(range(128 * c + 1, 128 * c + 129))

        emG = np.zeros((T, 2 * ESLOT), np.float32)
        for i, s in enumerate(s_list):
            emG[bidx // 2, (bidx % 2) * ESLOT + i] = em32[bidx, s, tags[:, s]]
        trG = np.zeros((T, 2 * ESLOT), np.float32)
        for sl, i in enumerate(i_list):
            trG[bidx // 2, (bidx % 2) * ESLOT + sl] = trans[prev_all[:, i], cur_all[:, i]]

        in_maps.append(
            {
                "emT": emT,
                "trans": trans,
                "transW": transW0 if c == 0 else trans,
                "x0": _bf16(x0_0 if c == 0 else x0_u),
                "emG": _bf16(emG),
                "trG": _bf16(trG),
            }
        )
    return in_maps


_NC_CACHE: list = []


def kernel(emissions: np.ndarray, tags: np.ndarray, transitions: np.ndarray) -> np.ndarray:
    if not _NC_CACHE:
        _NC_CACHE.append(_build_kernel())
    nc = _NC_CACHE[0]

    in_maps = _host_prep(emissions, tags, transitions)
    kernel._last_in_maps = in_maps
    results = run_bass_kernel_spmd(nc, in_maps, core_ids=list(range(NCORES))).results

    const = (S - 1) * PRE * math.log(2.0) - 10000.0
    partition = np.zeros(B, np.float64)
    emitsum = np.zeros(B, np.float64)
    transsum = np.zeros(B, np.float64)
    bidx = np.arange(B)
    for c in range(NCORES):
        r = results[c]
        z = r["lnz"].reshape(NSNAP, B).astype(np.float64)
        eD = z[7] if c == 7 else z[8]
        partition += (z[4] - z[0]) + (z[5] - z[1]) + (z[6] - z[2]) + (eD - z[3])
        gE = r["goldE"].astype(np.float64)
        emitsum += gE[bidx // 2, bidx % 2]
        gT = r["goldT"].astype(np.float64)
        transsum += gT[bidx // 2, bidx % 2]
    partition += const
    loss = (partition - emitsum - transsum).mean()
    return np.array(loss, dtype=np.float32)
